# revision 1
# baseline (speedup 1.0000x reference)
"""GCNHead Trainium2 kernel (8-core SPMD).

Math (matches reference):
  deg = bincount(dst)+1 (self loops);  dinv = deg^-1/2
  agg[n] = sum_{e: dst=n} dinv[src] * x[src]   (+ self loop)
  h = (dinv[n] * agg[n]) @ W_gcn + b_gcn
  out = leaky_relu(h, 0.2);  pooled = segment_max(out, batch)
  result = pooled @ (W_fc / sigma_max(W_fc)).T + b_fc

Sharding: 8 whole graphs per core (batch is sorted -> graphs are contiguous
node ranges). Per graph, nodes are assigned to padded "slots" (GCAP slots per
graph, tiles of 64 slots) with edge-count balancing across tiles. Edges are
bucketed by destination tile and source parity (pair gather), padded to
128-lane blocks. Device: x~ = dinv*x (bf16) written to DRAM; dma_gather
fetches 256B node-pair rows; a one-hot [128 edges x 64 slots] built on DVE
feeds PE matmuls that accumulate per-tile PSUM; tail = dinv_dst scale,
transpose, W_gcn transform, bias+leaky, segment max, spectral-norm FC.

Host does integer-only preprocessing (sharding, bucketing, index tables,
degree counts via bincount); all float math runs on device.
"""
import sys

sys.path.insert(0, "/opt/trn_rl_repo")

import math
import os
import numpy as np
import ml_dtypes

import concourse.bass as bass
import concourse.mybir as mybir
import concourse.tile as tile
from concourse import bacc
from concourse.masks import make_identity

BF16 = mybir.dt.bfloat16
F32 = mybir.dt.float32
I16 = mybir.dt.int16

NCORES = 8
SENT = 20000.0  # dst_local sentinel (never matches iota 0..63)


# ----------------------------------------------------------------------------
# Host preprocessing (integers only)
# ----------------------------------------------------------------------------
def _preprocess(x, edge_index, batch, num_graphs):
    N, D = x.shape
    B = int(num_graphs)
    src = np.asarray(edge_index[0], dtype=np.int64)
    dst = np.asarray(edge_index[1], dtype=np.int64)
    batch = np.asarray(batch, dtype=np.int64)

    deg = np.bincount(dst, minlength=N).astype(np.int64) + 1  # + self loop

    # graph -> node range (batch sorted)
    counts_g = np.bincount(batch, minlength=B)
    starts_g = np.concatenate([[0], np.cumsum(counts_g)])

    GPC = math.ceil(B / NCORES)  # graphs per core
    # balance graphs across cores by total edge weight (snake over sorted)
    gw = np.add.reduceat(deg, starts_g[:-1]) if N else counts_g
    gw = np.where(counts_g > 0, gw, 0)
    order = np.argsort(-gw, kind="stable")
    core_graphs = [[] for _ in range(NCORES)]
    loads = np.zeros(NCORES)
    for g in order:
        c = int(np.argmin([loads[i] + (1e18 if len(core_graphs[i]) >= GPC else 0)
                           for i in range(NCORES)]))
        core_graphs[c].append(int(g))
        loads[c] += gw[g]
    for c in range(NCORES):
        core_graphs[c] += [-1] * (GPC - len(core_graphs[c]))

    GCAP = 64 * max(1, math.ceil(counts_g.max() / 64))
    TPG = GCAP // 64          # tiles per graph
    TT = GPC * TPG            # tiles per core
    S = TT * 64               # slots per core

    # --- slot assignment: per graph, balance node degree across TPG bins ---
    node_slot = np.full(N, -1, dtype=np.int64)   # slot within its core
    node_core = np.full(N, -1, dtype=np.int64)
    deg_slot = np.ones((NCORES, S), dtype=np.float32)
    for c in range(NCORES):
        for gi, g in enumerate(core_graphs[c]):
            if g < 0:
                continue
            nodes = np.arange(starts_g[g], starts_g[g + 1])
            if len(nodes) == 0:
                continue
            nd = deg[nodes]
            ordn = np.argsort(-nd, kind="stable")
            binload = np.zeros(TPG, dtype=np.int64)
            binfill = np.zeros(TPG, dtype=np.int64)
            for i in ordn:
                masked = np.where(binfill < 64, binload, np.iinfo(np.int64).max)
                b = int(np.argmin(masked))
                slot = gi * GCAP + b * 64 + binfill[b]
                node_slot[nodes[i]] = slot
                node_core[nodes[i]] = c
                deg_slot[c, slot] = nd[i]
                binfill[b] += 1
                binload[b] += nd[i]

    # --- edges (incl self loops) bucketed per (core, tile, parity) ---
    loop = np.arange(N, dtype=np.int64)
    esrc = np.concatenate([src, loop])
    edst = np.concatenate([dst, loop])
    ecore = node_core[edst]
    eslot = node_slot[edst]
    etile = eslot >> 6
    edl = (eslot & 63).astype(np.int64)
    epar = (esrc & 1).astype(np.int64)

    # per-core lists of (tile, parity) buckets; order edges by key
    counts = np.zeros((NCORES, TT, 2), dtype=np.int64)
    per_core_order = []
    for c in range(NCORES):
        sel = np.where(ecore == c)[0]
        k = etile[sel] * 2 + epar[sel]
        o = np.argsort(k, kind="stable")
        sel = sel[o]
        per_core_order.append(sel)
        cnt = np.bincount(k[o], minlength=TT * 2)
        counts[c] = cnt.reshape(TT, 2)

    # class capacity = max over cores, rounded to 128
    cap = ((counts.max(axis=0) + 127) // 128) * 128      # [TT, 2]
    cap = np.maximum(cap, 0)
    blocks = cap // 128                                   # [TT, 2]
    TOTBLK = int(blocks.sum())
    TOTPOS = TOTBLK * 128

    # global block layout: tiles ascending, class even then odd
    class_off = np.zeros((TT, 2), dtype=np.int64)        # position offsets
    pos = 0
    for t in range(TT):
        for q in range(2):
            class_off[t, q] = pos
            pos += cap[t, q]

    # tables
    idx_tab = np.zeros((NCORES, TOTPOS), dtype=np.int64)
    dstl_tab = np.full((NCORES, TOTPOS), SENT, dtype=np.float32)
    for c in range(NCORES):
        sel = per_core_order[c]
        k = etile[sel] * 2 + epar[sel]
        # position within class = running index per class
        cstart = np.concatenate([[0], np.cumsum(np.bincount(k, minlength=TT * 2))])
        within = np.arange(len(sel)) - cstart[k]
        gpos = class_off.reshape(-1)[k] + within
        idx_tab[c, gpos] = esrc[sel] >> 1
        dstl_tab[c, gpos] = edl[sel]

    # idx table SBUF layout [128, TOTPOS/16]: flat i -> [i%16 (+16r), i//16]
    idx16 = idx_tab.astype(np.int16).reshape(NCORES, TOTPOS // 16, 16)
    idx16 = np.ascontiguousarray(idx16.transpose(0, 2, 1))           # [NC,16,P/16]
    idx128 = np.tile(idx16, (1, 8, 1))                               # [NC,128,...]
    # dst_local SBUF layout [128, TOTBLK]: flat i -> [i%128, i//128]
    dstl128 = np.ascontiguousarray(
        dstl_tab.reshape(NCORES, TOTBLK, 128).transpose(0, 2, 1)
    ).astype(ml_dtypes.bfloat16)

    # node-major deg [128, ceil(NP/128)] for the x~ prescale
    NP = ((N + 255) // 256) * 256          # pad to even multiple of 128
    degn = np.ones(NP, dtype=np.float32)
    degn[:N] = deg
    degn128 = np.ascontiguousarray(degn.reshape(NP // 128, 128).T)

    # slot-major deg [64, TT]
    degs = np.ones((NCORES, 64, TT), dtype=np.float32)
    for c in range(NCORES):
        degs[c] = deg_slot[c].reshape(TT, 64).T

    x_pad = np.zeros((NP, D), dtype=ml_dtypes.bfloat16)
    x_pad[:N] = np.asarray(x, dtype=np.float32).astype(ml_dtypes.bfloat16)

    dims = dict(N=N, D=D, B=B, GPC=GPC, GCAP=GCAP, TPG=TPG, TT=TT, S=S, NP=NP,
                TOTBLK=TOTBLK, TOTPOS=TOTPOS,
                blocks=tuple(map(tuple, blocks)),
                kmax=tuple(map(tuple, counts.max(axis=0))))
    tables = dict(idx=idx128, dstl=dstl128, degn=degn128, degs=degs,
                  x_pad=x_pad, core_graphs=core_graphs)
    return dims, tables


# ----------------------------------------------------------------------------
# Device program
# ----------------------------------------------------------------------------
def _build_program(dims):
    D = dims["D"]
    TT, TPG, GPC, GCAP = dims["TT"], dims["TPG"], dims["GPC"], dims["GCAP"]
    NP, TOTBLK, TOTPOS = dims["NP"], dims["TOTBLK"], dims["TOTPOS"]
    blocks = dims["blocks"]
    kmax = dims["kmax"]
    NPT = NP // 128            # node tiles
    S = dims["S"]

    nc = bacc.Bacc("TRN2", target_bir_lowering=False, debug=False,
                   num_swdge_queues=4)
    x_d = nc.dram_tensor("x", [NP, D], BF16, kind="ExternalInput")
    idx_d = nc.dram_tensor("idx", [128, TOTPOS // 16], I16, kind="ExternalInput")
    dstl_d = nc.dram_tensor("dstl", [128, TOTBLK], BF16, kind="ExternalInput")
    degn_d = nc.dram_tensor("degn", [128, NPT], F32, kind="ExternalInput")
    degs_d = nc.dram_tensor("degs", [64, TT], F32, kind="ExternalInput")
    wgcn_d = nc.dram_tensor("wgcn", [D, D], F32, kind="ExternalInput")
    bgcn_d = nc.dram_tensor("bgcn", [D, 1], F32, kind="ExternalInput")
    wfc_d = nc.dram_tensor("wfc", [D, D], F32, kind="ExternalInput")
    bfc_d = nc.dram_tensor("bfc", [D, 1], F32, kind="ExternalInput")
    out_d = nc.dram_tensor("out", [D, GPC], F32, kind="ExternalOutput")

    # max blocks in any 8-tile bank-batch (sizes the gather/one-hot tiles)
    GMAX = 0
    _pos = 0
    _tb = []
    for t in range(TT):
        _tb.append(_pos)
        _pos += blocks[t][0] + blocks[t][1]
    _tb.append(_pos)
    for b0 in range(0, TT, 8):
        nb = min(8, TT - b0)
        GMAX = max(GMAX, _tb[b0 + nb] - _tb[b0])

    with tile.TileContext(nc) as tc:
        with (
            tc.tile_pool(name="consts", bufs=1) as cp,
            tc.tile_pool(name="dram", bufs=1, space="DRAM") as dp,
        ):
            # ---------------- constants / tables ----------------
            idx_t = cp.tile([128, TOTPOS // 16], I16)
            nc.sync.dma_start(out=idx_t[:], in_=idx_d[:])
            dstl_t = cp.tile([128, TOTBLK], BF16)
            nc.sync.dma_start(out=dstl_t[:], in_=dstl_d[:])
            degn_t = cp.tile([128, NPT], F32)
            nc.sync.dma_start(out=degn_t[:], in_=degn_d[:])
            degs_t = cp.tile([64, TT], F32)
            nc.sync.dma_start(out=degs_t[:], in_=degs_d[:])
            wgcn_t = cp.tile([D, D], F32)
            nc.sync.dma_start(out=wgcn_t[:], in_=wgcn_d[:])
            bgcn_t = cp.tile([D, 1], F32)
            nc.sync.dma_start(out=bgcn_t[:], in_=bgcn_d[:])
            wfc_t = cp.tile([D, D], F32)
            nc.sync.dma_start(out=wfc_t[:], in_=wfc_d[:])
            bfc_t = cp.tile([D, 1], F32)
            nc.sync.dma_start(out=bfc_t[:], in_=bfc_d[:])

            iota_t = cp.tile([128, 64], BF16)
            nc.gpsimd.iota(iota_t[:], pattern=[[1, 64]], base=0,
                           channel_multiplier=0,
                           allow_small_or_imprecise_dtypes=True)
            ident_t = cp.tile([128, 128], F32)
            make_identity(nc, ident_t[:])

            # dinv tables
            dinvn_t = cp.tile([128, NPT], F32)
            nc.vector.reciprocal(dinvn_t[:], degn_t[:])
            nc.scalar.activation(dinvn_t[:], dinvn_t[:],
                                 mybir.ActivationFunctionType.Sqrt)
            dinvs_t = cp.tile([64, TT], F32)
            nc.vector.reciprocal(dinvs_t[:], degs_t[:])
            nc.scalar.activation(dinvs_t[:], dinvs_t[:],
                                 mybir.ActivationFunctionType.Sqrt)

            REPEAT = int(os.environ.get("GNN_REPEAT", "1"))
            for _it in range(REPEAT):
                # ---------------- phase 1: x~ = dinv * x (bf16, DRAM) --------
                xs_d = dp.tile([NP, D], BF16)
                P1T = 56                     # node tiles per chunk
                p1 = tc.alloc_tile_pool(name=f"phase1_{_it}", bufs=3)
                for t0 in range(0, NPT, P1T):
                    tl = min(P1T, NPT - t0)
                    xt = p1.tile([128, P1T, D], BF16, tag="xt")
                    nc.sync.dma_start(
                        out=xt[:, :tl, :],
                        in_=x_d[t0 * 128:(t0 + tl) * 128, :].rearrange(
                            "(t p) d -> p t d", p=128),
                    )
                    nc.vector.tensor_tensor(
                        out=xt[:, :tl, :], in0=xt[:, :tl, :],
                        in1=dinvn_t[:, t0:t0 + tl].to_broadcast([128, tl, D]),
                        op=mybir.AluOpType.mult,
                    )
                    nc.sync.dma_start(
                        out=xs_d[t0 * 128:(t0 + tl) * 128, :].rearrange(
                            "(t p) d -> p t d", p=128),
                        in_=xt[:, :tl, :],
                    )
                xs_pairs = xs_d[:].rearrange("(v two) d -> v (two d)", two=2)

                STAGE = int(os.environ.get("GNN_STAGE", "9"))
                _QRR = [0]
                # ---------------- phase 2: gather + scatter ------------------
                # bank-batches of up to 8 tiles
                hT = cp.tile([64, S], F32)
                # precompute per-tile block ranges
                tile_blk0 = []
                pos = 0
                for t in range(TT):
                    tile_blk0.append((pos, blocks[t][0], blocks[t][1]))
                    pos += blocks[t][0] + blocks[t][1]

                gp = tc.alloc_tile_pool(name=f"gath_{_it}", bufs=2)
                ohp = tc.alloc_tile_pool(name=f"oh_{_it}", bufs=2)
                tp = tc.alloc_tile_pool(name=f"tail_{_it}", bufs=2)
                pa = tc.alloc_tile_pool(name=f"psum_acc_{_it}", bufs=2, space="PSUM")
                pm = tc.alloc_tile_pool(name=f"psum_misc_{_it}", bufs=2, space="PSUM")
                for b0 in (range(0, TT, 8) if STAGE >= 2 else []):
                    nb = min(8, TT - b0)
                    blk0 = tile_blk0[b0][0]
                    blk1 = (tile_blk0[b0 + nb][0] if b0 + nb < TT else TOTBLK)
                    nblk = blk1 - blk0
                    if nblk > 0:
                        gat = gp.tile([128, GMAX, 128], BF16, tag="gat")
                        for g0 in range(0, nblk, 8):
                            gl = min(8, nblk - g0)
                            nc.gpsimd.dma_gather(
                                out_ap=gat[:, g0:g0 + gl, :],
                                in_ap=xs_pairs,
                                idxs_ap=idx_t[:, (blk0 + g0) * 8:(blk0 + g0 + gl) * 8],
                                num_idxs=gl * 128,
                                num_idxs_reg=gl * 128,
                                elem_size=128,
                                queue_num=_QRR[0],
                            )
                            _QRR[0] = (_QRR[0] + 1) % 4
                        if STAGE < 3:
                            continue
                        oh = ohp.tile([128, GMAX, 64], BF16, tag="oh")
                        nc.vector.tensor_tensor(
                            out=oh[:, :nblk, :],
                            in0=dstl_t[:, blk0:blk1].to_broadcast([128, nblk, 64]),
                            in1=iota_t[:, :64].rearrange(
                                "p (j m) -> p j m", j=1).to_broadcast([128, nblk, 64]),
                            op=mybir.AluOpType.is_equal,
                        )
                    acc = pa.tile([64, 8, 64], F32, tag="acc")
                    if nblk == 0:
                        nc.vector.memset(acc[:], 0.0)
                    for ti in range(nb):
                        t = b0 + ti
                        base, be, bo = tile_blk0[t]
                        nblks_t = be + bo
                        first = True
                        for j in range(nblks_t):
                            q = 0 if j < be else 1
                            jj = j if j < be else j - be
                            k = 128
                            if jj == (be if q == 0 else bo) - 1:
                                k = kmax[t][q] - 128 * jj
                                k = 128 if k <= 0 else k
                            lb = base - blk0 + j
                            nc.tensor.matmul(
                                out=acc[:, ti, :],
                                lhsT=oh[:k, lb, :],
                                rhs=gat[:k, lb, 64 * q:64 * q + 64],
                                start=first,
                                stop=(j == nblks_t - 1),
                            )
                            first = False
                        if nblks_t == 0 and nblk > 0:
                            nc.vector.memset(acc[:, ti, :], 0.0)

                    if STAGE < 4:
                        continue
                    # tail per batch: scale by dinv_dst, transpose, transform
                    agg_sc = tp.tile([64, 8, 64], F32, tag="aggsc")
                    nc.vector.tensor_tensor(
                        out=agg_sc[:, :nb, :], in0=acc[:, :nb, :],
                        in1=dinvs_t[:, b0:b0 + nb].to_broadcast([64, nb, 64]),
                        op=mybir.AluOpType.mult,
                    )
                    trp = pm.tile([64, 8, 64], F32, tag="trp")
                    for ti in range(nb):
                        nc.tensor.transpose(
                            out=trp[:, ti, :], in_=agg_sc[:, ti, :],
                            identity=ident_t[:64, :64],
                        )
                    aggT = tp.tile([64, 8, 64], F32, tag="aggT")
                    nc.scalar.copy(out=aggT[:, :nb, :], in_=trp[:, :nb, :])
                    hps = pm.tile([64, 8 * 64], F32, tag="hps")
                    nc.tensor.matmul(
                        out=hps[:, :nb * 64],
                        lhsT=wgcn_t[:],
                        rhs=aggT[:, :nb, :].rearrange("p a b -> p (a b)"),
                        start=True, stop=True,
                    )
                    nc.scalar.activation(
                        out=hT[:, b0 * 64:(b0 + nb) * 64], in_=hps[:, :nb * 64],
                        func=mybir.ActivationFunctionType.Identity,
                        bias=bgcn_t[:],
                    )
                    # leaky relu on this batch's columns
                    lk = tp.tile([64, 8 * 64], F32, tag="lk")
                    nc.vector.tensor_scalar_mul(
                        lk[:, :nb * 64], hT[:, b0 * 64:(b0 + nb) * 64], 0.2)
                    nc.vector.tensor_tensor(
                        out=hT[:, b0 * 64:(b0 + nb) * 64],
                        in0=hT[:, b0 * 64:(b0 + nb) * 64],
                        in1=lk[:, :nb * 64], op=mybir.AluOpType.max)

                for _pool in (pm, pa, tp, ohp, gp, p1):
                    _pool.release()

                # ---------------- pooling ----------------
                if STAGE < 5:
                    pooledT = None
                pooledT = cp.tile([64, GPC], F32)
                for g in (range(GPC) if STAGE >= 5 else []):
                    nc.vector.tensor_reduce(
                        out=pooledT[:, g:g + 1],
                        in_=hT[:, g * GCAP:(g + 1) * GCAP],
                        axis=mybir.AxisListType.X,
                        op=mybir.AluOpType.max,
                    )

                # ---------------- spectral norm sigma(W_fc) ----------------
                out_sb0 = cp.tile([D, GPC], F32, tag="outsb0")
                if STAGE >= 5:
                    pf = tc.alloc_tile_pool(name=f"psum_fc_{_it}", bufs=1, space="PSUM")
                    mp = pf.tile([D, D], F32, tag="mp")
                    nc.tensor.matmul(out=mp[:], lhsT=wfc_t[:], rhs=wfc_t[:],
                                     start=True, stop=True)
                    m1_sb = cp.tile([D, D], F32, tag="m1sb")
                    nc.scalar.copy(out=m1_sb[:], in_=mp[:])
                    cur = m1_sb
                    for _ in range(6):  # M^64
                        mp2 = pf.tile([D, D], F32, tag="mp")
                        nc.tensor.matmul(out=mp2[:], lhsT=cur[:], rhs=cur[:],
                                         start=True, stop=True)
                        nxt = cp.tile([D, D], F32, tag=f"m{_}")
                        nc.scalar.copy(out=nxt[:], in_=mp2[:])
                        cur = nxt
                    ones_c = cp.tile([D, 1], F32)
                    nc.vector.memset(ones_c[:], 1.0)
                    ones_r = cp.tile([1, D], F32)
                    nc.vector.memset(ones_r[:], 1.0)
                    vp = pf.tile([D, 1], F32, tag="vp")
                    nc.tensor.matmul(out=vp[:], lhsT=cur[:], rhs=ones_c[:],
                                     start=True, stop=True)
                    v_sb = cp.tile([D, 1], F32)
                    nc.scalar.copy(out=v_sb[:], in_=vp[:])
                    wp = pf.tile([D, 1], F32, tag="vp")
                    nc.tensor.matmul(out=wp[:], lhsT=m1_sb[:], rhs=v_sb[:],
                                     start=True, stop=True)
                    w_sb = cp.tile([D, 1], F32)
                    nc.scalar.copy(out=w_sb[:], in_=wp[:])
                    nump = pf.tile([1, 1], F32, tag="sc")
                    nc.tensor.matmul(out=nump[:], lhsT=v_sb[:], rhs=w_sb[:],
                                     start=True, stop=True)
                    denp = pf.tile([1, 1], F32, tag="sc")
                    nc.tensor.matmul(out=denp[:], lhsT=v_sb[:], rhs=v_sb[:],
                                     start=True, stop=True)
                    num_sb = cp.tile([1, 1], F32, tag="num")
                    den_sb = cp.tile([1, 1], F32, tag="den")
                    nc.vector.tensor_copy(out=num_sb[:], in_=nump[:])
                    nc.vector.tensor_copy(out=den_sb[:], in_=denp[:])
                    rinv = cp.tile([1, 1], F32, tag="rinv")
                    nc.vector.reciprocal(rinv[:], num_sb[:])
                    nc.vector.tensor_tensor(out=rinv[:], in0=rinv[:], in1=den_sb[:],
                                            op=mybir.AluOpType.mult)
                    nc.scalar.activation(rinv[:], rinv[:],
                                         mybir.ActivationFunctionType.Sqrt)
                    sp = pf.tile([D, 1], F32, tag="vp")
                    nc.tensor.matmul(out=sp[:], lhsT=ones_r[:], rhs=rinv[:],
                                     start=True, stop=True)
                    s_col = cp.tile([D, 1], F32)
                    nc.scalar.copy(out=s_col[:], in_=sp[:])

                    # W_fc^T, scaled by 1/sigma
                    wtp = pf.tile([D, D], F32, tag="mp")
                    nc.tensor.transpose(out=wtp[:], in_=wfc_t[:],
                                        identity=ident_t[:D, :D])
                    wfcT = cp.tile([D, D], F32)
                    nc.vector.tensor_scalar_mul(wfcT[:], wtp[:], s_col[:])

                    # FC: outT = (W/sigma) @ pooledT + b_fc
                    op_ = pf.tile([D, GPC], F32, tag="op")
                    nc.tensor.matmul(out=op_[:], lhsT=wfcT[:], rhs=pooledT[:],
                                     start=True, stop=True)
                    out_sb = cp.tile([D, GPC], F32)
                    nc.scalar.activation(out=out_sb[:], in_=op_[:],
                                         func=mybir.ActivationFunctionType.Identity,
                                         bias=bfc_t[:])
                    nc.sync.dma_start(out=out_d[:], in_=out_sb[:])
                    pf.release()
                else:
                    nc.vector.memset(out_sb0[:], 0.0)
                    nc.sync.dma_start(out=out_d[:], in_=out_sb0[:])


    nc.compile()
    return nc


# ----------------------------------------------------------------------------
# Cached executor (compile once per dims signature)
# ----------------------------------------------------------------------------
_CACHE = {}


class _Exec:
    def __init__(self, dims):
        self.dims = dims
        self.nc = _build_program(dims)

    def run(self, in_maps):
        from concourse.bass_utils import run_bass_kernel_spmd
        res = run_bass_kernel_spmd(self.nc, in_maps, list(range(NCORES)))
        return [r["out"] for r in res.results]


def _get_exec(dims):
    key = repr(sorted(dims.items()))
    if key not in _CACHE:
        _CACHE[key] = _Exec(dims)
    return _CACHE[key]


def _make_in_maps(dims, tables, W_gcn, b_gcn, W_fc, b_fc):
    wgcn = np.asarray(W_gcn, dtype=np.float32)
    bgcn = np.asarray(b_gcn, dtype=np.float32).reshape(-1, 1)
    wfc = np.asarray(W_fc, dtype=np.float32)
    bfc = np.asarray(b_fc, dtype=np.float32).reshape(-1, 1)
    in_maps = []
    for c in range(NCORES):
        in_maps.append({
            "x": tables["x_pad"],
            "idx": np.ascontiguousarray(tables["idx"][c]),
            "dstl": np.ascontiguousarray(tables["dstl"][c]),
            "degn": tables["degn"],
            "degs": np.ascontiguousarray(tables["degs"][c]),
            "wgcn": wgcn,
            "bgcn": bgcn,
            "wfc": wfc,
            "bfc": bfc,
        })
    return in_maps


def kernel(x, W_gcn, b_gcn, W_fc, b_fc, edge_index, batch, num_graphs):
    dims, tables = _preprocess(x, edge_index, batch, num_graphs)
    ex = _get_exec(dims)
    in_maps = _make_in_maps(dims, tables, W_gcn, b_gcn, W_fc, b_fc)
    outs = ex.run(in_maps)
    B = dims["B"]
    D = dims["D"]
    result = np.zeros((B, D), dtype=np.float32)
    for c in range(NCORES):
        o = np.asarray(outs[c], dtype=np.float32)  # [D, GPC]
        for gi, g in enumerate(tables["core_graphs"][c]):
            if g >= 0:
                result[g] = o[:, gi]
    return result


# ----------------------------------------------------------------------------
# Reusable jitted runner (for steady-state timing): mirrors
# bass2jax.run_bass_via_pjrt's multi-core path but keeps the jitted callable.
# ----------------------------------------------------------------------------
def _build_jit(nc):
    import jax
    import numpy as _np
    from jax.sharding import Mesh, PartitionSpec
    from jax.experimental.shard_map import shard_map
    from concourse import bass2jax
    from concourse import mybir as _mb

    bass2jax.install_neuronx_cc_hook()
    in_names, out_names, out_avals, zero_outs = [], [], [], []
    partition_name = (nc.partition_id_tensor.name
                      if nc.partition_id_tensor else None)
    for alloc in nc.m.functions[0].allocations:
        if not isinstance(alloc, _mb.MemoryLocationSet):
            continue
        name = alloc.memorylocations[0].name
        if alloc.kind == "ExternalInput":
            if name != partition_name:
                in_names.append(name)
        elif alloc.kind == "ExternalOutput":
            out_names.append(name)
            shape = tuple(alloc.tensor_shape)
            dtype = _mb.dt.np(alloc.dtype)
            out_avals.append(jax.core.ShapedArray(shape, dtype))
            zero_outs.append(_np.zeros(shape, dtype))
    n_params = len(in_names)
    all_in = list(in_names) + list(out_names)
    if partition_name is not None:
        all_in.append(partition_name)

    def _body(*args):
        operands = list(args)
        if partition_name is not None:
            operands.append(bass2jax.partition_id_tensor())
        outs = bass2jax._bass_exec_p.bind(
            *operands,
            out_avals=tuple(out_avals),
            in_names=tuple(all_in),
            out_names=tuple(out_names),
            lowering_input_output_aliases=(),
            sim_require_finite=True,
            sim_require_nnan=True,
            nc=nc,
        )
        return tuple(outs)

    devices = jax.devices()[:NCORES]
    mesh = Mesh(np.asarray(devices), ("core",))
    in_specs = (PartitionSpec("core"),) * (n_params + len(out_names))
    out_specs = (PartitionSpec("core"),) * len(out_names)
    donate = tuple(range(n_params, n_params + len(out_names)))
    fn = jax.jit(
        shard_map(_body, mesh=mesh, in_specs=in_specs, out_specs=out_specs,
                  check_rep=False),
        donate_argnums=donate, keep_unused=True,
    )
    return fn, in_names, out_names, zero_outs


def time_exec(ex, in_maps, reps=24, warmup=4):
    """Median per-iteration device time via back-to-back dispatch."""
    import jax
    import time as _t
    fn, in_names, out_names, zero_outs = _build_jit(ex.nc)
    concat = [np.concatenate([np.asarray(in_maps[c][n]) for c in range(NCORES)],
                             axis=0) for n in in_names]
    dev_in = [jax.device_put(a) for a in concat]
    for a in dev_in:
        a.block_until_ready()

    def zouts():
        return [np.concatenate([z] * NCORES, axis=0) for z in zero_outs]

    def run_n(n):
        outs = None
        t0 = _t.perf_counter()
        for _ in range(n):
            outs = fn(*dev_in, *zouts())
        for o in outs:
            o.block_until_ready()
        return _t.perf_counter() - t0

    run_n(warmup)
    t1 = run_n(reps // 2)
    t2 = run_n(reps)
    per_iter = (t2 - t1) / (reps - reps // 2)
    return per_iter * 1e9

